# revision 19
# baseline (speedup 1.0000x reference)
"""Trainium2 kernel for nn_BinaryDiffRow.

Math: y = x @ base_t + (x * coeff) @ S, with S = unpack_signs(mask) in {-1,+1}.
Folded on host into a single matmul: y = x @ W_eff,
W_eff = base_t + coeff[:,None] * S, shipped pre-scaled by 512 so the fp8
chunks sit in e4m3's normal range (the PSUM drain multiplies by 1/512).

Sharding: hybrid 2-way tensor-parallel (output columns) x 4-way data-parallel
(tokens). Core j (tp = j%2, dp = j//2) owns output cols [2048*tp, +2048) and
tokens [2048*dp, +2048). The 2048-col W slab stays resident in SBUF. Within a
token tile the matmul loop runs oc-outer/k-inner so consecutive matmuls hit
the same PSUM bank in 28-long streaks — per-matmul bank rotation measured
~60 cycles/matmul of turnaround, while the per-matmul LDWEIGHTS this ordering
costs is hidden by the PE's reorder window.

Precision: the first 4 super-chunks of K (1024 of 4096) run as fp8-e4m3
DoubleRow matmuls (K=256 per instruction, 2x PE throughput); the remaining
3072 K run in bf16. Measured deterministic rel err vs the f32 reference is
0.0192 (gate 2e-2); the fp8 fraction is capped by accuracy, not speed.

PSUM: 4 output-chunk tags x 2 bufs = all 8 banks; DVE drains each bank to a
bf16 tile (x1/512) which DMAs out; host re-assembles and upcasts to f32.
"""

import os
import sys

import numpy as np

for _p in ("/opt/trn_rl_repo",):
    if _p not in sys.path and os.path.isdir(_p):
        sys.path.insert(0, _p)

import ml_dtypes  # noqa: E402

B, S, IN, OUT = 4, 2048, 4096, 4096
NTOK = B * S  # 8192
NCORES = 8
P = 128
NBITS = 32

NTP = 2                      # tensor-parallel ways (output cols)
NDP = NCORES // NTP          # data-parallel ways (tokens)
OUT_SH = OUT // NTP          # 2048 cols per core
TOK_SH = NTOK // NDP         # 2048 tokens per core
NOC = OUT_SH // 512          # matmuls per stationary x chunk
WSCALE = 512.0

F8 = 4                       # fp8-e4m3 DoubleRow super-chunks of 256 k each
KCB = (IN - 256 * F8) // P   # bf16 k-chunks
TT = TOK_SH // P             # token tiles per core


def build_bass(f8=F8, repeat_phase2=1, x_bufs=3):
    """Single-core Bass program (SPMD: all 8 cores run this)."""
    import concourse.mybir as mybir
    import concourse.tile as tile
    from concourse import bacc
    from contextlib import ExitStack

    kcb = (IN - 256 * f8) // P
    tt = TT

    nc = bacc.Bacc("TRN2")
    dt = mybir.dt
    Alu = mybir.AluOpType

    xb = nc.dram_tensor("xb", (tt, P, kcb, P), dt.bfloat16, kind="ExternalInput")
    wb = nc.dram_tensor("wb", (kcb, P, OUT_SH), dt.bfloat16, kind="ExternalInput")
    if f8:
        xh = nc.dram_tensor("xh", (tt, P, f8, 2, P), dt.float8e4, kind="ExternalInput")
        w8 = nc.dram_tensor("w8", (P, f8, 2, OUT_SH), dt.float8e4, kind="ExternalInput")
    y = nc.dram_tensor("y", (TOK_SH, OUT_SH), dt.bfloat16, kind="ExternalOutput")

    with ExitStack() as ctx:
        tc = ctx.enter_context(tile.TileContext(nc))
        wpool = ctx.enter_context(tc.tile_pool(name="w", bufs=1))
        xpool = ctx.enter_context(tc.tile_pool(name="x", bufs=x_bufs))
        opool = ctx.enter_context(tc.tile_pool(name="out", bufs=2))
        pspool = ctx.enter_context(tc.tile_pool(name="ps", bufs=2, space="PSUM"))

        def body():
            # Queue assignment: W owns the sync queue, x rides gpsimd,
            # y-outs get scalar to themselves. Keeping W's queue free of
            # y-out instructions matters in the steady state: y(t15) can
            # only retire at iteration end, and anything enqueued behind
            # it stalls the next iteration's first matmuls.
            if f8:
                w8_sb = wpool.tile([P, f8, 2, OUT_SH], dt.float8e4, tag="w8")
                for c in range(f8):
                    nc.sync.dma_start(w8_sb[:, c, :, :], w8[:, c, :, :])
            wb_sb = []
            for k in range(kcb):
                wt = wpool.tile([P, OUT_SH], dt.bfloat16, tag=f"wb{k}")
                nc.sync.dma_start(wt[:], wb[k])
                wb_sb.append(wt)

            for t in range(tt):
                if f8:
                    xh_t = xpool.tile(
                        [P, f8, 2, P], dt.float8e4, tag="xh", name=f"xh_{t}"
                    )
                    nc.gpsimd.dma_start(xh_t[:], xh[t])
                xb_t = xpool.tile([P, kcb, P], dt.bfloat16, tag="xb", name=f"xb_{t}")
                nc.gpsimd.dma_start(xb_t[:], xb[t])
                ps = [
                    pspool.tile([P, 512], dt.float32, tag=f"ps{oc}", name=f"ps{oc}_{t}")
                    for oc in range(NOC)
                ]
                # oc-outer / k-inner: consecutive matmuls hit the SAME psum
                # bank in long streaks (the per-matmul bank-switch turnaround
                # is the leading suspect for the ~60cyc/MM overhead every
                # ordering with per-MM bank rotation has shown). Per-bank
                # k-order is unchanged, so the output stays bitwise identical.
                for oc in range(NOC):
                    if f8:
                        for c in range(f8):
                            nc.tensor.matmul(
                                ps[oc][:],
                                lhsT=xh_t[:, c, :, :],
                                rhs=w8_sb[:, c, :, oc * 512 : (oc + 1) * 512],
                                start=(c == 0),
                                stop=(kcb == 0 and c == f8 - 1),
                                perf_mode=mybir.MatmulPerfMode.DoubleRow,
                            )
                    for k in range(kcb):
                        nc.tensor.matmul(
                            ps[oc][:],
                            lhsT=xb_t[:, k, :],
                            rhs=wb_sb[k][:, oc * 512 : (oc + 1) * 512],
                            start=(f8 == 0 and k == 0),
                            stop=(k == kcb - 1),
                        )
                for oc in range(NOC):
                    o_sb = opool.tile(
                        [P, 512], dt.bfloat16, tag=f"o{oc}", name=f"o{oc}_{t}"
                    )
                    nc.vector.tensor_scalar(
                        o_sb[:], ps[oc][:], 1.0 / WSCALE, None, Alu.mult
                    )
                    nc.scalar.dma_start(
                        y[t * P : (t + 1) * P, oc * 512 : (oc + 1) * 512], o_sb[:]
                    )

        if repeat_phase2 == 1:
            body()
        elif repeat_phase2 < 0:  # python-unrolled repeat (sim-only slope probe)
            for _ in range(-repeat_phase2):
                body()
        else:
            # benchmarking: repeat the idempotent body in a HW loop so one
            # NEFF execution amortizes the ~60-100ms axon dispatch overhead
            with tc.For_i(0, repeat_phase2, 1):
                body()

    nc.finalize()
    dedupe_ldweights(nc)
    return nc


def dedupe_ldweights(nc):
    """Drop the 2nd+ of consecutive identical PE Ldweights. If the redundant
    LDW carries only semaphore updates (no waits), delete it and fold its
    increments into the next PE instruction; otherwise keep it as a NoOp
    holding the sync_info."""
    import concourse.mybir as mybir

    def wsig(inst):
        return str(inst.ins[0])

    n_del = n_nop = 0
    for fn in nc.m.functions:
        for blk in fn.blocks:
            last_ldw_sig = None
            new_insts = []
            pending_updates = None
            for inst in blk.instructions:
                eng = getattr(inst, "engine", None)
                if eng == mybir.EngineType.PE and pending_updates is not None:
                    si = inst.sync_info
                    if si is None:
                        inst.sync_info = mybir.SyncInfo(
                            on_wait=[], on_update=list(pending_updates)
                        )
                    else:
                        merged = list(si.on_update)
                        for upd in pending_updates:
                            for m in merged:
                                if m.id == upd.id and m.update_mode == upd.update_mode:
                                    m.update_value = m.update_value + upd.update_value
                                    break
                            else:
                                merged.append(upd)
                        si.on_update = merged
                    pending_updates = None
                if eng != mybir.EngineType.PE:
                    new_insts.append(inst)
                    continue
                if isinstance(inst, mybir.InstLdweights):
                    sig = wsig(inst)
                    if sig == last_ldw_sig:
                        si = inst.sync_info
                        waits = list(si.on_wait) if si else []
                        upds = list(si.on_update) if si else []
                        if not waits:
                            if upds:
                                pending_updates = upds
                            n_del += 1
                            continue
                        new_insts.append(
                            mybir.InstNoOp(
                                name=inst.name,
                                engine=mybir.EngineType.PE,
                                ins=[],
                                outs=[],
                                sync_info=inst.sync_info,
                            )
                        )
                        n_nop += 1
                        continue
                    last_ldw_sig = sig
                elif isinstance(inst, mybir.InstMatmult):
                    if getattr(inst, "ldweights", False):
                        last_ldw_sig = None
                new_insts.append(inst)
            assert pending_updates is None, "trailing folded updates lost"
            blk.instructions[:] = new_insts
    return n_del, n_nop


def make_in_maps(x, base_t, coeff, mask, f8=F8):
    kcb = (IN - 256 * f8) // P
    tt = TT
    e4 = ml_dtypes.float8_e4m3  # TRN FP8_EXP4 flavor (max 240)
    b16 = ml_dtypes.bfloat16

    bits = ((mask[:, :, None].astype(np.int64) >> np.arange(NBITS)) & 1).reshape(
        IN, OUT
    )
    W = (
        base_t.astype(np.float32)
        + coeff.astype(np.float32)[:, None] * (bits.astype(np.float32) * 2.0 - 1.0)
    ) * WSCALE

    xT = np.ascontiguousarray(x.reshape(-1, IN).T.astype(np.float32))  # (IN, NTOK)

    in_maps = []
    for j in range(NCORES):
        tp, dp = j % NTP, j // NTP
        cols = slice(tp * OUT_SH, (tp + 1) * OUT_SH)
        toks = slice(dp * TOK_SH, (dp + 1) * TOK_SH)
        xTj = xT[:, toks]  # (IN, TOK_SH)

        m = {}
        m["xb"] = np.ascontiguousarray(
            xTj[256 * f8 :].reshape(kcb, P, tt, P).transpose(2, 1, 0, 3).astype(b16)
        )
        m["wb"] = np.ascontiguousarray(
            W[256 * f8 :, cols].reshape(kcb, P, OUT_SH).astype(b16)
        )
        if f8:
            m["xh"] = np.ascontiguousarray(
                xTj[: 256 * f8]
                .astype(e4)
                .reshape(f8, 2, P, tt, P)
                .transpose(3, 2, 0, 1, 4)
            )
            m["w8"] = np.ascontiguousarray(
                W[: 256 * f8, cols].astype(e4).reshape(f8, 2, P, OUT_SH).transpose(2, 0, 1, 3)
            )
        in_maps.append(m)
    return in_maps


def gather_outputs(per_core):
    """Assemble per-core bf16 y slabs into the full (NTOK, OUT) f32 matrix."""
    Y = np.empty((NTOK, OUT), np.float32)
    for j in range(NCORES):
        tp, dp = j % NTP, j // NTP
        Y[dp * TOK_SH : (dp + 1) * TOK_SH, tp * OUT_SH : (tp + 1) * OUT_SH] = (
            np.asarray(per_core[j]["y"]).astype(np.float32)
        )
    return Y


_CACHED = {}


def kernel(x, base_t, coeff, mask):
    from concourse.bass_utils import run_bass_kernel_spmd

    x = np.asarray(x, dtype=np.float32)
    base_t = np.asarray(base_t, dtype=np.float32)
    coeff = np.asarray(coeff, dtype=np.float32)
    mask = np.asarray(mask, dtype=np.int32)

    if "nc" not in _CACHED:
        _CACHED["nc"] = build_bass()
    nc = _CACHED["nc"]
    in_maps = make_in_maps(x, base_t, coeff, mask)
    res = run_bass_kernel_spmd(nc, in_maps, core_ids=list(range(NCORES)))
    return gather_outputs(res.results).reshape(B, S, OUT)


if __name__ == "__main__":
    rng = np.random.default_rng(0)
    x = rng.standard_normal((B, S, IN), dtype=np.float32)
    base_t = (rng.standard_normal((IN, OUT), dtype=np.float32) * 0.02).astype(np.float32)
    coeff = (rng.random(IN, dtype=np.float32) * 0.01).astype(np.float32)
    mask = rng.integers(0, 2**31 - 1, size=(IN, OUT // NBITS), dtype=np.int32)
    y = kernel(x=x, base_t=base_t, coeff=coeff, mask=mask)
    bits = ((mask[:, :, None].astype(np.int64) >> np.arange(NBITS)) & 1).reshape(IN, OUT)
    W = base_t + coeff[:, None] * (bits * 2.0 - 1.0).astype(np.float32)
    yref = x.reshape(-1, IN) @ W
    err = np.abs(y.reshape(-1, OUT) - yref).max() / np.abs(yref).max()
    print("y", y.shape, y.dtype, "rel err", err)
